# revision 1
# baseline (speedup 1.0000x reference)
"""NemotronH Mamba2 mixer on 8 Trainium2 cores (Bass/Tile).

Sharding: tensor-parallel over heads/groups. Core c owns group c =
16 heads (= 1024 gate/x channels, 128 B + 128 C state channels, 16 dt).
in_proj rows and out_proj columns are sharded accordingly; out_proj is
row-parallel over the contraction, partials are combined on the host.

Per-core dataflow (seq superblocks of 512, SSD chunks of 128):
  in_proj (fp32r matmul, weights pre-tiled for single-DMA loads)
  -> depthwise conv taps on DVE + SiLU
  -> Mamba2 chunked SSD: per-head decay matrices built with a PE
     broadcast matmul (indicator x cs), Ydiag + Yoff accumulated in one
     PSUM group per head
  -> gated group RMSNorm -> out_proj (fp32r) -> partial [4096, 2048].
"""

import numpy as np

import concourse.bass as bass
import concourse.mybir as mybir
from concourse import bacc
from concourse.tile import TileContext
from concourse.bass_utils import run_bass_kernel_spmd

F32 = mybir.dt.float32
F32R = mybir.dt.float32r
AF = mybir.ActivationFunctionType
ALU = mybir.AluOpType

# Model dims
H_SIZE = 4096
NH = 128
HD = 64
SS = 128
KCONV = 4
NG = 8
CHUNK = 128
INTER = NH * HD                 # 8192
CONV_DIM = INTER + 2 * NG * SS  # 10240
PROJ = INTER + CONV_DIM + NH    # 18560
DT_MIN, DT_MAX = 0.001, 100.0
EPS = 1e-5
GROUP = INTER // NG             # 1024

# Sharding / tiling
N_CORES = 8
S = 2048
HL = NH // N_CORES              # 16 local heads
CLOC = HL * HD                  # 1024 local gate/x channels
NSB = 4                         # seq superblocks
SB = S // NSB                   # 512
NCPB = SB // CHUNK              # 4 chunks per superblock
NCH = S // CHUNK                # 16 chunks
NF = 19                         # in_proj f-tiles (2432 = 19*128, padded)
NK1 = H_SIZE // 128             # 32 k-tiles for in_proj
NK2 = CLOC // 128               # 8 k-tiles for out_proj
NM2 = H_SIZE // 128             # 32 m-tiles for out_proj
NEGM = -1e30

# log1p(u)/u on [0,1], Chebyshev-fit degree 12 (max fp32 err ~1.1e-7)
LOG1P_C = [0.9999999999815061, -0.4999999935552795, 0.33333295899388315,
           -0.2499913901062215, 0.19989602251462296, -0.1659083573590588,
           0.1392317246686566, -0.1130135727826319, 0.08261769871302305,
           -0.04960969557400616, 0.021956439674455992, -0.006180556818034449,
           0.0008159022224092772]

_CACHE = {}


def r32(ap):
    return ap.bitcast(F32R)


def build_nc():
    nc = bacc.Bacc(None, target_bir_lowering=False)

    # hidden, pre-tiled: [sb, half, 128, 16*512] (per-partition contiguous)
    hids = nc.declare_dram_parameter("hids", [NSB, 2, 128, 16 * SB], F32,
                                     isOutput=False)
    # in_proj weights, pre-tiled per f-tile: [f, half, 128, 16*128]
    w1f = nc.declare_dram_parameter("w1f", [NF, 2, 128, 16 * 128], F32,
                                    isOutput=False)
    # out_proj weights, pre-tiled per m-tile: [m, 128, 8*128]
    w2m = nc.declare_dram_parameter("w2m", [NM2, 128, NK2 * 128], F32,
                                    isOutput=False)
    convw = nc.declare_dram_parameter("convw", [128, 10 * KCONV], F32,
                                      isOutput=False)
    convb = nc.declare_dram_parameter("convb", [128, 10], F32, isOutput=False)
    dtbias = nc.declare_dram_parameter("dtbias", [HL, 1], F32, isOutput=False)
    acol = nc.declare_dram_parameter("acol", [HL, 1], F32, isOutput=False)
    dbc = nc.declare_dram_parameter("dbc", [128, HL], F32, isOutput=False)
    negmask = nc.declare_dram_parameter("negmask", [128, 128], F32,
                                        isOutput=False)
    ident = nc.declare_dram_parameter("ident", [128, 128], F32, isOutput=False)
    e127 = nc.declare_dram_parameter("e127", [128, 1], F32, isOutput=False)
    outp = nc.declare_dram_parameter("outp", [NM2, 4, 128, 512], F32,
                                     isOutput=True)

    with TileContext(nc) as tc:
        with tc.tile_pool(name="const", bufs=1) as cp, \
             tc.tile_pool(name="dram", bufs=1, space="DRAM") as dp:
            id_sb = cp.tile([128, 128], F32, tag="id")
            nm_sb = cp.tile([128, 128], F32, tag="nm")
            dbc_sb = cp.tile([128, HL], F32, tag="dbc")
            cw_sb = cp.tile([128, 10 * KCONV], F32, tag="cw")
            cb_sb = cp.tile([128, 10], F32, tag="cb")
            dtb_sb = cp.tile([HL, 1], F32, tag="dtb")
            a_sb = cp.tile([HL, 1], F32, tag="acol")
            e127_sb = cp.tile([128, 1], F32, tag="e127")
            ones16 = cp.tile([HL, CHUNK], F32, tag="ones16")
            st_sb = cp.tile([128, HL * HD], F32, tag="state")
            nc.sync.dma_start(out=id_sb[:], in_=ident[:])
            nc.sync.dma_start(out=nm_sb[:], in_=negmask[:])
            nc.sync.dma_start(out=dbc_sb[:], in_=dbc[:])
            nc.sync.dma_start(out=cw_sb[:], in_=convw[:])
            nc.sync.dma_start(out=cb_sb[:], in_=convb[:])
            nc.sync.dma_start(out=dtb_sb[:], in_=dtbias[:])
            nc.sync.dma_start(out=a_sb[:], in_=acol[:])
            nc.sync.dma_start(out=e127_sb[:], in_=e127[:])
            nc.vector.memset(ones16[:], 1.0)
            nc.vector.memset(st_sb[:], 0.0)

            _main_phase(nc, tc, hids, w1f, id_sb, nm_sb, dbc_sb,
                        cw_sb, cb_sb, dtb_sb, a_sb, e127_sb, ones16,
                        st_sb, w2m, outp)

    nc.compile()
    return nc


def _main_phase(nc, tc, hids, w1f, id_sb, nm_sb, dbc_sb,
                cw_sb, cb_sb, dtb_sb, a_sb, e127_sb, ones16,
                st_sb, w2m, outp):
    with tc.tile_pool(name="hid", bufs=2) as hidp, \
         tc.tile_pool(name="w1", bufs=3) as w1p, \
         tc.tile_pool(name="gq", bufs=2) as gqp, \
         tc.tile_pool(name="conv", bufs=1) as convp, \
         tc.tile_pool(name="dtl", bufs=1) as dtp, \
         tc.tile_pool(name="dtr", bufs=2) as dtrp, \
         tc.tile_pool(name="cch", bufs=2) as cchp, \
         tc.tile_pool(name="chunk", bufs=2) as chp, \
         tc.tile_pool(name="chunk1", bufs=1) as ch1p, \
         tc.tile_pool(name="w2", bufs=2) as w2p, \
         tc.tile_pool(name="oev", bufs=2) as oevp, \
         tc.tile_pool(name="heads", bufs=1) as hp, \
         tc.tile_pool(name="psA", bufs=1, space="PSUM") as psA, \
         tc.tile_pool(name="psS", bufs=1, space="PSUM") as psS, \
         tc.tile_pool(name="psT", bufs=1, space="PSUM") as psT, \
         tc.tile_pool(name="psY", bufs=2, space="PSUM") as psY:

        # convcat: 10 conv channel tiles (8 x, 1 B, 1 C), each 3 halo + 512
        ccat = convp.tile([128, 10 * (SB + 3)], F32, tag="ccat")
        for t in range(10):
            nc.vector.memset(ccat[:, t * (SB + 3):t * (SB + 3) + 3], 0.0)

        pending_out = []

        def emit_outproj(m, qst, sbq):
            w2 = w2p.tile([128, NK2 * 128], F32R, tag="w2")
            nc.sync.dma_start(out=w2[:], in_=r32(w2m[m]))
            acc = psA.tile([128, 512], F32, tag="ipacc")
            for kt in range(NK2):
                nc.tensor.matmul(
                    acc[:], w2[:, kt * 128:(kt + 1) * 128],
                    qst[:, kt * SB:kt * SB + SB],
                    start=(kt == 0), stop=(kt == NK2 - 1))
            ev = oevp.tile([128, 512], F32, tag="oev")
            nc.scalar.copy(ev[:], acc[:])
            nc.sync.dma_start(out=outp[m, sbq], in_=ev[:])

        for sb in range(NSB):
            # ---------------- in_proj for this superblock ----------------
            halves = []
            for khalf in range(2):
                hid_h = hidp.tile([128, 16 * SB], F32R, tag="hid")
                nc.sync.dma_start(out=hid_h[:], in_=r32(hids[sb, khalf]))
                halves.append(hid_h)

            gate_sb = gqp.tile([128, 8 * SB], F32, tag="gq")
            dtraw = dtrp.tile([HL, SB], F32, tag="dtraw")

            # halo copies must read previous superblock before overwrite
            if sb > 0:
                for t in range(10):
                    base = t * (SB + 3)
                    nc.vector.tensor_copy(
                        ccat[:, base:base + 3], ccat[:, base + SB:base + SB + 3])

            def emit_ftile(f, gate_sb=gate_sb, dtraw=dtraw, halves=halves):
                w1h = []
                for khalf in range(2):
                    w1t_ = w1p.tile([128, 16 * 128], F32R, tag="w1")
                    nc.sync.dma_start(out=w1t_[:], in_=r32(w1f[f, khalf]))
                    w1h.append(w1t_)
                acc = psA.tile([128, SB], F32, tag="ipacc")
                for k in range(NK1):
                    nc.tensor.matmul(
                        acc[:],
                        w1h[k // 16][:, (k % 16) * 128:(k % 16 + 1) * 128],
                        halves[k // 16][:, (k % 16) * SB:(k % 16 + 1) * SB],
                        start=(k == 0), stop=(k == NK1 - 1))
                if f < 8:
                    nc.scalar.copy(gate_sb[:, f * SB:(f + 1) * SB], acc[:])
                elif f < 18:
                    t = f - 8
                    base = t * (SB + 3)
                    nc.scalar.copy(ccat[:, base + 3:base + 3 + SB], acc[:])
                else:
                    nc.scalar.copy(dtraw[:, :], acc[:HL, :])

            for f in [18] + list(range(18)):
                emit_ftile(f)
                for _ in range(2):
                    if pending_out:
                        emit_outproj(*pending_out.pop(0))

            nc.scalar.activation(gate_sb[:], gate_sb[:], AF.Silu)

            # ---------------- dt pipeline ----------------
            # softplus(z) = relu(z) + log1p(exp(-|z|)); log1p via poly
            # (no Softplus/Ln activation table on gen3)
            uu = dtp.tile([HL, SB], F32, tag="uu")
            pp = dtrp.tile([HL, SB], F32, tag="pp")
            dtsp = dtraw  # in-place: relu(z) overwrites z
            cs = pp       # reuse pp once the poly is folded in
            nc.scalar.activation(dtraw[:], dtraw[:], AF.Identity,
                                 bias=dtb_sb[:, 0:1])
            # uu = exp(min(z, -z)) = exp(-|z|)
            nc.vector.tensor_scalar(uu[:], dtraw[:], -1.0, None, ALU.mult)
            nc.vector.tensor_tensor(uu[:], uu[:], dtraw[:], ALU.min)
            nc.scalar.activation(uu[:], uu[:], AF.Exp)
            # Horner for q(u) = log1p(u)/u
            nc.vector.tensor_scalar(pp[:], uu[:], LOG1P_C[-1], LOG1P_C[-2],
                                    ALU.mult, ALU.add)
            for cidx in range(len(LOG1P_C) - 3, -1, -1):
                nc.vector.tensor_tensor(pp[:], pp[:], uu[:], ALU.mult)
                nc.vector.tensor_scalar(pp[:], pp[:], LOG1P_C[cidx], None,
                                        ALU.add)
            nc.vector.tensor_tensor(pp[:], pp[:], uu[:], ALU.mult)
            relu_t = uu  # uu dead; use as relu scratch
            nc.scalar.activation(relu_t[:], dtraw[:], AF.Relu)
            nc.vector.tensor_tensor(dtsp[:], relu_t[:], pp[:], ALU.add)
            nc.vector.tensor_scalar(dtsp[:], dtsp[:], DT_MIN, DT_MAX,
                                    ALU.max, ALU.min)
            dA = uu  # reuse again (relu scratch is dead)
            nc.vector.tensor_scalar(dA[:], dtsp[:], a_sb[:, 0:1], None,
                                    ALU.mult)
            for cl in range(NCPB):
                nc.vector.tensor_tensor_scan(
                    cs[:, cl * CHUNK:(cl + 1) * CHUNK],
                    ones16[:], dA[:, cl * CHUNK:(cl + 1) * CHUNK],
                    0.0, ALU.mult, ALU.add)

            # ---------------- SSD chunks ----------------
            qstage = gqp.tile([128, NK2 * SB], F32R, tag="gq")

            def emit_conv(cl):
                xc = cchp.tile([128, 8 * CHUNK], F32, tag="xc")
                bcs = ch1p.tile([128, CHUNK], F32, tag="bc")
                ccs = ch1p.tile([128, CHUNK], F32, tag="cc")
                for t in range(10):
                    base = t * (SB + 3) + cl * CHUNK
                    dst = (xc[:, t * CHUNK:(t + 1) * CHUNK] if t < 8
                           else (bcs[:] if t == 8 else ccs[:]))
                    nc.vector.tensor_scalar(
                        dst, ccat[:, base:base + CHUNK],
                        cw_sb[:, t * KCONV:t * KCONV + 1], cb_sb[:, t:t + 1],
                        ALU.mult, ALU.add)
                    for j in range(1, KCONV):
                        nc.vector.scalar_tensor_tensor(
                            dst, ccat[:, base + j:base + j + CHUNK],
                            cw_sb[:, t * KCONV + j:t * KCONV + j + 1], dst,
                            ALU.mult, ALU.add)
                    nc.scalar.activation(dst, dst, AF.Silu)
                return xc, bcs, ccs

            for cl in range(NCPB):
                ch = sb * NCPB + cl
                csl = slice(cl * CHUNK, (cl + 1) * CHUNK)
                xc, bcs, ccs = emit_conv(cl)

                # gate transpose + SiLU
                gps = psT.tile([128, CLOC], F32, tag="trans")
                for t in range(8):
                    nc.tensor.transpose(
                        gps[:, t * 128:(t + 1) * 128],
                        gate_sb[:, t * SB + cl * CHUNK:t * SB + (cl + 1) * CHUNK],
                        id_sb[:])
                silg = ch1p.tile([128, CLOC], F32, tag="silg")
                nc.scalar.copy(silg[:], gps[:])

                # small transposes: csT, dtT
                pT = psS.tile([128, 128], F32, tag="small")
                nc.tensor.transpose(pT[:, :HL], cs[:, csl], id_sb[:HL, :HL])
                csT = chp.tile([128, HL], F32, tag="csT")
                negcsT = chp.tile([128, HL], F32, tag="negcsT")
                nc.scalar.copy(csT[:], pT[:, :HL])
                nc.scalar.mul(negcsT[:], pT[:, :HL], -1.0)

                pT2 = psS.tile([128, 128], F32, tag="small")
                nc.tensor.transpose(pT2[:, :HL], dtsp[:, csl], id_sb[:HL, :HL])
                dtT = chp.tile([128, HL], F32, tag="dtT")
                nc.scalar.copy(dtT[:], pT2[:, :HL])

                # cs at chunk end, broadcast across partitions (PE matmul)
                pT3 = psS.tile([128, 128], F32, tag="small")
                e127b = bass.AP(tensor=e127_sb.tensor,
                                offset=e127_sb[:].offset,
                                ap=[[e127_sb[:].ap[0][0], 128], [0, 128]])
                nc.tensor.matmul(pT3[:, :HL], e127b, csT[:],
                                 start=True, stop=True)
                cdbc = chp.tile([128, HL], F32, tag="cdbc")
                decT = chp.tile([128, HL], F32, tag="decT")
                nc.scalar.activation(cdbc[:], pT3[:, :HL], AF.Exp)
                nc.vector.tensor_tensor(decT[:], pT3[:, :HL], csT[:],
                                        ALU.subtract)
                nc.scalar.activation(decT[:], decT[:], AF.Exp)
                ddt = chp.tile([128, HL], F32, tag="ddt")
                nc.vector.tensor_tensor(ddt[:], dtT[:], decT[:], ALU.mult)

                # x transpose -> xT, then xdt / xdd
                xps = psT.tile([128, CLOC], F32, tag="trans")
                for t in range(8):
                    nc.tensor.transpose(
                        xps[:, t * 128:(t + 1) * 128],
                        xc[:, t * CHUNK:(t + 1) * CHUNK], id_sb[:])
                xT = ch1p.tile([128, CLOC], F32, tag="xT")
                nc.scalar.copy(xT[:], xps[:])
                xdt = ch1p.tile([128, CLOC], F32, tag="xdt")
                xdd = ch1p.tile([128, CLOC], F32R, tag="xdd")
                for h in range(HL):
                    hs = slice(h * HD, (h + 1) * HD)
                    nc.vector.tensor_scalar(
                        xdt[:, hs], xT[:, hs], dtT[:, h:h + 1], None, ALU.mult)
                for h in range(HL):
                    hs = slice(h * HD, (h + 1) * HD)
                    nc.vector.tensor_scalar(
                        xdd[:, hs], xT[:, hs], ddt[:, h:h + 1], None, ALU.mult)

                # B chunk transposed (B_LN)
                pbt = psS.tile([128, 128], F32, tag="small")
                nc.tensor.transpose(pbt[:], bcs[:], id_sb[:])
                bln = chp.tile([128, 128], F32R, tag="bln")
                nc.scalar.copy(bln[:], pbt[:])

                # Gram^T = B C^T in [s, l]; evicted to SBUF
                gram_ps = psS.tile([128, 128], F32, tag="small")
                nc.tensor.matmul(gram_ps[:], bcs[:], ccs[:],
                                 start=True, stop=True)
                gram = ch1p.tile([128, 128], F32, tag="gram")
                nc.scalar.copy(gram[:], gram_ps[:])

                # per-head decay matrices in groups of 4 heads
                y_ps = psY.tile([128, CLOC], F32, tag="yo")
                for g in range(HL // 4):
                    pb4 = psS.tile([128, 512], F32, tag="small")
                    for j in range(4):
                        h = 4 * g + j
                        idcol = id_sb[:HL, h:h + 1]
                        indh = bass.AP(tensor=idcol.tensor,
                                       offset=idcol.offset,
                                       ap=[[idcol.ap[0][0], HL], [0, 128]])
                        nc.tensor.matmul(pb4[:, j * 128:(j + 1) * 128], indh,
                                         cs[:, csl], start=True, stop=True)
                    epb4 = hp.tile([128, 512], F32, tag="epb")
                    nc.scalar.activation(epb4[:], pb4[:], AF.Exp)
                    seg4 = hp.tile([128, 512], F32, tag="seg")
                    for j in range(4):
                        h = 4 * g + j
                        nc.vector.scalar_tensor_tensor(
                            seg4[:, j * 128:(j + 1) * 128],
                            pb4[:, j * 128:(j + 1) * 128],
                            negcsT[:, h:h + 1], nm_sb[:], ALU.add, ALU.add)
                    nc.scalar.activation(seg4[:], seg4[:], AF.Exp)
                    gram_b = bass.AP(tensor=gram.tensor, offset=gram[:].offset,
                                     ap=[gram[:].ap[0], [0, 4], [1, 128]])
                    ccs_b = bass.AP(tensor=ccs.tensor, offset=ccs[:].offset,
                                    ap=[ccs[:].ap[0], [0, 4], [1, 128]])
                    s4 = seg4[:].rearrange("p (j l) -> p j l", j=4)
                    e4 = epb4[:].rearrange("p (j l) -> p j l", j=4)
                    nc.vector.tensor_tensor(s4, s4, gram_b, ALU.mult)
                    nc.vector.tensor_tensor(e4, e4, ccs_b, ALU.mult)
                    for j in range(4):
                        h = 4 * g + j
                        hs = slice(h * HD, (h + 1) * HD)
                        nc.tensor.matmul(
                            y_ps[:, hs], seg4[:, j * 128:(j + 1) * 128],
                            xdt[:, hs], start=True, stop=False)
                        nc.tensor.matmul(
                            y_ps[:, hs], epb4[:, j * 128:(j + 1) * 128],
                            st_sb[:, hs], start=False, stop=True)

                # states for this chunk
                s_ps = psY.tile([128, CLOC], F32, tag="yo")
                for half in range(2):
                    hsl = slice(half * 512, (half + 1) * 512)
                    nc.tensor.matmul(
                        s_ps[:, hsl], bln[:], xdd[:, hsl],
                        start=True, stop=True)

                # y = (Ydiag + Yoff) + D*x ; state update
                y_sb = ch1p.tile([128, CLOC], F32, tag="ysb")
                for h in range(HL):
                    hs = slice(h * HD, (h + 1) * HD)
                    nc.vector.scalar_tensor_tensor(
                        y_sb[:, hs], xT[:, hs], dbc_sb[:, h:h + 1],
                        y_ps[:, hs], ALU.mult, ALU.add)
                for h in range(HL):
                    hs = slice(h * HD, (h + 1) * HD)
                    nc.vector.scalar_tensor_tensor(
                        st_sb[:, hs], st_sb[:, hs], cdbc[:, h:h + 1],
                        s_ps[:, hs], ALU.mult, ALU.add)

                # gate + group RMSNorm
                nc.vector.tensor_tensor(y_sb[:], y_sb[:], silg[:], ALU.mult)
                ssum = ch1p.tile([128, 1], F32, tag="ssum")
                # Square's main output is discarded into xdd (scratch)
                nc.scalar.activation(xdd[:], y_sb[:], AF.Square,
                                     accum_out=ssum[:, 0:1])
                nc.vector.tensor_scalar(ssum[:], ssum[:], 1.0 / GROUP, EPS,
                                        ALU.mult, ALU.add)
                rstd = chp.tile([128, 1], F32, tag="rstd")
                tnew = chp.tile([128, 1], F32, tag="tnew")
                nc.scalar.activation(tnew[:], ssum[:], AF.Sqrt)
                nc.vector.reciprocal(rstd[:], tnew[:])
                normed = ch1p.tile([128, CLOC], F32, tag="normed")
                nc.vector.tensor_scalar(
                    normed[:], y_sb[:], rstd[:, 0:1], None, ALU.mult)

                # transpose normed -> [c, s] and stage out to DRAM
                nps = psT.tile([128, CLOC], F32, tag="trans")
                for t in range(8):
                    nc.tensor.transpose(
                        nps[:, t * 128:(t + 1) * 128],
                        normed[:, t * 128:(t + 1) * 128], id_sb[:])
                qdst = qstage[:].rearrange(
                    "p (t s) -> p t s", t=NK2)[:, :, cl * 128:(cl + 1) * 128]
                nsrc = nps[:].rearrange("p (t s) -> p t s", t=NK2)
                nc.scalar.copy(qdst, nsrc)

            # out_proj m-blocks are deferred and interleaved into the
            # next superblock's in_proj f-loop (shared psA rotation)
            pending_out.extend((m, qstage, sb) for m in range(NM2))

        while pending_out:
            emit_outproj(*pending_out.pop(0))


def prepare_in_maps(hidden_states, in_proj_w, conv_w, conv_b, dt_bias, D,
                    norm_w, out_proj_w):
    hidT = np.ascontiguousarray(hidden_states.reshape(S, H_SIZE).T)
    # [half, kk, r, sb, c] -> [sb, half, r, kk, c]
    hids = np.ascontiguousarray(
        hidT.reshape(2, 16, 128, NSB, SB).transpose(3, 0, 2, 1, 4)
        .reshape(NSB, 2, 128, 16 * SB))
    negmask = np.where(np.arange(128)[None, :] >= np.arange(128)[:, None],
                       np.float32(0.0), np.float32(NEGM)).astype(np.float32)
    ident = np.eye(128, dtype=np.float32)
    e127 = np.zeros((128, 1), np.float32)
    e127[127, 0] = 1.0
    in_maps = []
    for c in range(N_CORES):
        gsl = slice(CLOC * c, CLOC * (c + 1))
        xsl = slice(INTER + CLOC * c, INTER + CLOC * (c + 1))
        bsl = slice(2 * INTER + SS * c, 2 * INTER + SS * (c + 1))
        cslc = slice(2 * INTER + NG * SS + SS * c,
                     2 * INTER + NG * SS + SS * (c + 1))
        dsl = slice(INTER + CONV_DIM + HL * c, INTER + CONV_DIM + HL * (c + 1))
        w1 = np.concatenate([in_proj_w[gsl], in_proj_w[xsl], in_proj_w[bsl],
                             in_proj_w[cslc], in_proj_w[dsl]], axis=0)
        w1 = np.concatenate(
            [w1, np.zeros((NF * 128 - w1.shape[0], H_SIZE), np.float32)],
            axis=0)
        # W1T [4096, 2432]: [half, kk, r, f, fc] -> [f, half, r, kk, fc]
        w1f = np.ascontiguousarray(
            w1.T.reshape(2, 16, 128, NF, 128).transpose(3, 0, 2, 1, 4)
            .reshape(NF, 2, 128, 16 * 128))
        w2 = out_proj_w[:, gsl] * norm_w[gsl][None, :]  # norm_w folded
        # W2T [1024, 4096]: [kt, r, m, mc] -> [m, r, kt, mc]
        w2m = np.ascontiguousarray(
            w2.T.reshape(NK2, 128, NM2, 128).transpose(2, 1, 0, 3)
            .reshape(NM2, 128, NK2 * 128))
        conv_idx = np.concatenate([
            np.arange(CLOC * c, CLOC * (c + 1)),
            np.arange(INTER + SS * c, INTER + SS * (c + 1)),
            np.arange(INTER + NG * SS + SS * c,
                      INTER + NG * SS + SS * (c + 1))])
        cwl = conv_w[conv_idx, 0, :]          # [1280, 4]
        cbl = conv_b[conv_idx]                # [1280]
        convw = np.ascontiguousarray(
            cwl.reshape(10, 128, KCONV).transpose(1, 0, 2)
            .reshape(128, 10 * KCONV))
        convb = np.ascontiguousarray(cbl.reshape(10, 128).transpose(1, 0))
        hsl = slice(HL * c, HL * (c + 1))
        acol = -(np.arange(HL * c + 1, HL * (c + 1) + 1, dtype=np.float32))
        in_maps.append({
            "hids": hids,
            "w1f": w1f,
            "w2m": w2m,
            "convw": convw,
            "convb": convb,
            "dtbias": dt_bias[hsl].reshape(HL, 1).astype(np.float32),
            "acol": acol.reshape(HL, 1),
            "dbc": np.tile(D[hsl][None, :], (128, 1)).astype(np.float32),
            "negmask": negmask,
            "ident": ident,
            "e127": e127,
        })
    return in_maps


def get_nc():
    if "nc" not in _CACHE:
        _CACHE["nc"] = build_nc()
    return _CACHE["nc"]


def kernel(hidden_states, in_proj_w, conv_w, conv_b, dt_bias, D, norm_w,
           out_proj_w):
    nc = get_nc()
    in_maps = prepare_in_maps(
        np.asarray(hidden_states, np.float32),
        np.asarray(in_proj_w, np.float32),
        np.asarray(conv_w, np.float32), np.asarray(conv_b, np.float32),
        np.asarray(dt_bias, np.float32), np.asarray(D, np.float32),
        np.asarray(norm_w, np.float32), np.asarray(out_proj_w, np.float32))
    res = run_bass_kernel_spmd(nc, in_maps, list(range(N_CORES)))
    acc = np.zeros((H_SIZE, S), np.float64)
    for r in res.results:
        acc += r["outp"].transpose(0, 2, 1, 3).reshape(H_SIZE, S)
    return acc.T.astype(np.float32).reshape(1, S, H_SIZE)



# revision 2
# speedup vs baseline: 1.2199x; 1.2199x over previous
"""NemotronH Mamba2 mixer on 8 Trainium2 cores (Bass/Tile) — v2 (bf16).

Sharding: tensor-parallel over heads/groups. Core c owns group c =
16 heads (= 1024 gate/x channels, 128 B + 128 C state channels, 16 dt).
out_proj is row-parallel over the contraction; bf16 partials are
combined on the host.

v2 changes vs baseline: bf16 matmuls/transposes/elementwise everywhere
except the dt/cs chain (fp32), activation-table phasing (silu -> exp ->
sqrt once per superblock), conv taps at superblock granularity (4x DVE
mode), decay matrices built in a pre-pass with batched exps, D*x folded
into the per-head PSUM group via constant D*I stationary tiles.
"""

import numpy as np
import ml_dtypes

import concourse.bass as bass
import concourse.mybir as mybir
from concourse import bacc
from concourse.tile import TileContext
from concourse.bass_utils import run_bass_kernel_spmd

F32 = mybir.dt.float32
F32R = mybir.dt.float32r
BF16 = mybir.dt.bfloat16
AF = mybir.ActivationFunctionType
ALU = mybir.AluOpType
NPBF = ml_dtypes.bfloat16

# Model dims
H_SIZE = 4096
NH = 128
HD = 64
SS = 128
KCONV = 4
NG = 8
CHUNK = 128
INTER = NH * HD                 # 8192
CONV_DIM = INTER + 2 * NG * SS  # 10240
PROJ = INTER + CONV_DIM + NH    # 18560
DT_MIN, DT_MAX = 0.001, 100.0
EPS = 1e-5
GROUP = INTER // NG             # 1024

# Sharding / tiling
N_CORES = 8
S = 2048
HL = NH // N_CORES              # 16 local heads
CLOC = HL * HD                  # 1024 local gate/x channels
NSB = 4                         # seq superblocks
SB = S // NSB                   # 512
NCPB = SB // CHUNK              # 4 chunks per superblock
NF = 19                         # in_proj f-tiles (2432 = 19*128, padded)
NK1 = H_SIZE // 128             # 32 k-tiles for in_proj
NK2 = CLOC // 128               # 8 k-tiles for out_proj
NM2 = H_SIZE // 128             # 32 m-tiles for out_proj
NEGM = -1e30

# log1p(u)/u on [0,1], Chebyshev-fit degree 12 (max fp32 err ~1.1e-7)
LOG1P_C = [0.9999999999815061, -0.4999999935552795, 0.33333295899388315,
           -0.2499913901062215, 0.19989602251462296, -0.1659083573590588,
           0.1392317246686566, -0.1130135727826319, 0.08261769871302305,
           -0.04960969557400616, 0.021956439674455992, -0.006180556818034449,
           0.0008159022224092772]

_CACHE = {}


def r32(ap):
    return ap.bitcast(F32R)


def build_nc():
    nc = bacc.Bacc(None, target_bir_lowering=False)

    # hidden, pre-tiled: [sb, half, 128, 16*512] bf16
    hids = nc.declare_dram_parameter("hids", [NSB, 2, 128, 16 * SB], BF16,
                                     isOutput=False)
    # in_proj weights, pre-tiled per f-tile: [f, half, 128, 16*128] bf16
    w1f = nc.declare_dram_parameter("w1f", [NF, 2, 128, 16 * 128], BF16,
                                    isOutput=False)
    # out_proj weights, pre-tiled per m-tile: [m, 128, 8*128] bf16
    w2m = nc.declare_dram_parameter("w2m", [NM2, 128, NK2 * 128], BF16,
                                    isOutput=False)
    convw = nc.declare_dram_parameter("convw", [128, 10 * KCONV], F32,
                                      isOutput=False)
    convb = nc.declare_dram_parameter("convb", [128, 10], F32, isOutput=False)
    dtbias = nc.declare_dram_parameter("dtbias", [HL, 1], F32, isOutput=False)
    acol = nc.declare_dram_parameter("acol", [HL, 1], F32, isOutput=False)
    ddiag = nc.declare_dram_parameter("ddiag", [128, HL * 128], BF16,
                                      isOutput=False)
    negmask = nc.declare_dram_parameter("negmask", [128, 128], F32,
                                        isOutput=False)
    ident = nc.declare_dram_parameter("ident", [128, 128], F32, isOutput=False)
    identb = nc.declare_dram_parameter("identb", [128, 128], BF16,
                                       isOutput=False)
    e127 = nc.declare_dram_parameter("e127", [128, 1], F32, isOutput=False)
    outp = nc.declare_dram_parameter("outp", [NM2, 4, 128, 512], BF16,
                                     isOutput=True)

    with TileContext(nc) as tc:
        with tc.tile_pool(name="const", bufs=1) as cp:
            id_sb = cp.tile([128, 128], F32, tag="id")
            idb_sb = cp.tile([128, 128], BF16, tag="idb")
            nm_sb = cp.tile([128, 128], F32, tag="nm")
            ddiag_sb = cp.tile([128, HL * 128], BF16, tag="ddiag")
            cw_sb = cp.tile([128, 10 * KCONV], F32, tag="cw")
            cb_sb = cp.tile([128, 10], F32, tag="cb")
            dtb_sb = cp.tile([HL, 1], F32, tag="dtb")
            a_sb = cp.tile([HL, 1], F32, tag="acol")
            e127_sb = cp.tile([128, 1], F32, tag="e127")
            ones16 = cp.tile([HL, CHUNK], F32, tag="ones16")
            st_sb = cp.tile([128, HL * HD], BF16, tag="state")
            nc.sync.dma_start(out=id_sb[:], in_=ident[:])
            nc.sync.dma_start(out=idb_sb[:], in_=identb[:])
            nc.sync.dma_start(out=nm_sb[:], in_=negmask[:])
            nc.sync.dma_start(out=ddiag_sb[:], in_=ddiag[:])
            nc.sync.dma_start(out=cw_sb[:], in_=convw[:])
            nc.sync.dma_start(out=cb_sb[:], in_=convb[:])
            nc.sync.dma_start(out=dtb_sb[:], in_=dtbias[:])
            nc.sync.dma_start(out=a_sb[:], in_=acol[:])
            nc.sync.dma_start(out=e127_sb[:], in_=e127[:])
            nc.vector.memset(ones16[:], 1.0)
            nc.vector.memset(st_sb[:], 0.0)

            _main_phase(nc, tc, hids, w1f, id_sb, idb_sb, nm_sb, ddiag_sb,
                        cw_sb, cb_sb, dtb_sb, a_sb, e127_sb, ones16,
                        st_sb, w2m, outp)

    nc.compile()
    return nc


def _main_phase(nc, tc, hids, w1f, id_sb, idb_sb, nm_sb, ddiag_sb,
                cw_sb, cb_sb, dtb_sb, a_sb, e127_sb, ones16,
                st_sb, w2m, outp):
    with tc.tile_pool(name="hid", bufs=2) as hidp, \
         tc.tile_pool(name="w1", bufs=3) as w1p, \
         tc.tile_pool(name="gq", bufs=2) as gqp, \
         tc.tile_pool(name="conv", bufs=1) as convp, \
         tc.tile_pool(name="dtl", bufs=1) as dtp, \
         tc.tile_pool(name="dtr", bufs=2) as dtrp, \
         tc.tile_pool(name="dec", bufs=1) as decp, \
         tc.tile_pool(name="chunk", bufs=2) as chp, \
         tc.tile_pool(name="chunk1", bufs=1) as ch1p, \
         tc.tile_pool(name="w2", bufs=2) as w2p, \
         tc.tile_pool(name="oev", bufs=2) as oevp, \
         tc.tile_pool(name="psA", bufs=2, space="PSUM") as psA, \
         tc.tile_pool(name="psS", bufs=2, space="PSUM") as psS, \
         tc.tile_pool(name="psT", bufs=2, space="PSUM") as psT, \
         tc.tile_pool(name="psY", bufs=2, space="PSUM") as psY:

        # convcat: 10 conv channel tiles (8 x, 1 B, 1 C), each 3 halo + 512
        ccat = convp.tile([128, 10 * (SB + 3)], BF16, tag="ccat")
        for t in range(10):
            nc.vector.memset(ccat[:, t * (SB + 3):t * (SB + 3) + 3], 0.0)
        # conv+silu output: x tiles 0..7, B tile 8, C tile 9
        convo = convp.tile([128, 10 * SB], BF16, tag="convo")

        pending_out = []

        def emit_outproj(m, qst, sbq):
            w2 = w2p.tile([128, NK2 * 128], BF16, tag="w2")
            nc.sync.dma_start(out=w2[:], in_=w2m[m])
            acc = psA.tile([128, 512], F32, tag="ipacc")
            for kt in range(NK2):
                nc.tensor.matmul(
                    acc[:], w2[:, kt * 128:(kt + 1) * 128],
                    qst[:, kt * SB:kt * SB + SB],
                    start=(kt == 0), stop=(kt == NK2 - 1))
            ev = oevp.tile([128, 512], BF16, tag="oev")
            nc.scalar.copy(ev[:], acc[:])
            nc.sync.dma_start(out=outp[m, sbq], in_=ev[:])

        def drain_out(n):
            for _ in range(n):
                if pending_out:
                    emit_outproj(*pending_out.pop(0))

        for sb in range(NSB):
            # ---------------- in_proj for this superblock ----------------
            halves = []
            for khalf in range(2):
                hid_h = hidp.tile([128, 16 * SB], BF16, tag="hid")
                nc.sync.dma_start(out=hid_h[:], in_=hids[sb, khalf])
                halves.append(hid_h)

            gate_sb = gqp.tile([128, 8 * SB], BF16, tag="gq")
            dtraw = dtrp.tile([HL, SB], F32, tag="dtraw")

            # halo copies must read previous superblock before overwrite
            if sb > 0:
                for t in range(10):
                    base = t * (SB + 3)
                    nc.vector.tensor_copy(
                        ccat[:, base:base + 3], ccat[:, base + SB:base + SB + 3])

            def emit_ftile(f, gate_sb=gate_sb, dtraw=dtraw, halves=halves):
                w1h = []
                for khalf in range(2):
                    w1t_ = w1p.tile([128, 16 * 128], BF16, tag="w1")
                    nc.sync.dma_start(out=w1t_[:], in_=w1f[f, khalf])
                    w1h.append(w1t_)
                acc = psA.tile([128, SB], F32, tag="ipacc")
                for k in range(NK1):
                    nc.tensor.matmul(
                        acc[:],
                        w1h[k // 16][:, (k % 16) * 128:(k % 16 + 1) * 128],
                        halves[k // 16][:, (k % 16) * SB:(k % 16 + 1) * SB],
                        start=(k == 0), stop=(k == NK1 - 1))
                if f < 8:
                    # fused SiLU at eviction -> bf16 gate
                    nc.scalar.activation(
                        gate_sb[:, f * SB:(f + 1) * SB], acc[:], AF.Silu)
                elif f < 18:
                    t = f - 8
                    base = t * (SB + 3)
                    nc.scalar.copy(ccat[:, base + 3:base + 3 + SB], acc[:])
                else:
                    nc.scalar.copy(dtraw[:, :], acc[:HL, :])

            for f in [18] + list(range(18)):
                emit_ftile(f)

            # ---------------- conv (DVE, bf16 4x) + SiLU ----------------
            for t in range(10):
                base = t * (SB + 3)
                dst = convo[:, t * SB:(t + 1) * SB]
                nc.vector.tensor_scalar(
                    dst, ccat[:, base:base + SB],
                    cw_sb[:, t * KCONV:t * KCONV + 1], cb_sb[:, t:t + 1],
                    ALU.mult, ALU.add)
                for j in range(1, KCONV):
                    nc.vector.scalar_tensor_tensor(
                        dst, ccat[:, base + j:base + j + SB],
                        cw_sb[:, t * KCONV + j:t * KCONV + j + 1], dst,
                        ALU.mult, ALU.add)
            nc.scalar.activation(convo[:, :8 * SB], convo[:, :8 * SB], AF.Silu)
            nc.scalar.activation(convo[:, 8 * SB:], convo[:, 8 * SB:], AF.Silu)
            drain_out(2)

            # ---------------- dt pipeline (fp32, exp table from here) ----
            # softplus(z) = relu(z) + log1p(exp(-|z|)); log1p via poly
            uu = dtp.tile([HL, SB], F32, tag="uu")
            pp = dtrp.tile([HL, SB], F32, tag="pp")
            dtsp = dtraw  # in-place: relu(z) overwrites z
            cs = pp       # reuse pp once the poly is folded in
            nc.scalar.activation(dtraw[:], dtraw[:], AF.Identity,
                                 bias=dtb_sb[:, 0:1])
            # uu = exp(min(z, -z)) = exp(-|z|)
            nc.vector.tensor_scalar(uu[:], dtraw[:], -1.0, None, ALU.mult)
            nc.vector.tensor_tensor(uu[:], uu[:], dtraw[:], ALU.min)
            nc.scalar.activation(uu[:], uu[:], AF.Exp)
            # Horner for q(u) = log1p(u)/u
            nc.vector.tensor_scalar(pp[:], uu[:], LOG1P_C[-1], LOG1P_C[-2],
                                    ALU.mult, ALU.add)
            for cidx in range(len(LOG1P_C) - 3, -1, -1):
                nc.vector.tensor_tensor(pp[:], pp[:], uu[:], ALU.mult)
                nc.vector.tensor_scalar(pp[:], pp[:], LOG1P_C[cidx], None,
                                        ALU.add)
            nc.vector.tensor_tensor(pp[:], pp[:], uu[:], ALU.mult)
            relu_t = uu  # uu dead; use as relu scratch
            nc.scalar.activation(relu_t[:], dtraw[:], AF.Relu)
            nc.vector.tensor_tensor(dtsp[:], relu_t[:], pp[:], ALU.add)
            nc.vector.tensor_scalar(dtsp[:], dtsp[:], DT_MIN, DT_MAX,
                                    ALU.max, ALU.min)
            dA = uu  # reuse again (relu scratch is dead)
            nc.vector.tensor_scalar(dA[:], dtsp[:], a_sb[:, 0:1], None,
                                    ALU.mult)
            for cl in range(NCPB):
                nc.vector.tensor_tensor_scan(
                    cs[:, cl * CHUNK:(cl + 1) * CHUNK],
                    ones16[:], dA[:, cl * CHUNK:(cl + 1) * CHUNK],
                    0.0, ALU.mult, ALU.add)
            drain_out(2)

            # ------------ decay pre-pass for all 4 chunks (exp table) ----
            # per-chunk tiles, all bf16 except the fp32 cs-derived smalls
            csT4 = decp.tile([128, NCPB * HL], F32, tag="csT4")
            negcsT4 = decp.tile([128, NCPB * HL], F32, tag="negcsT4")
            dtT4 = decp.tile([128, NCPB * HL], F32, tag="dtT4")
            cdbc4 = decp.tile([128, NCPB * HL], F32, tag="cdbc4")
            ddt4 = decp.tile([128, NCPB * HL], F32, tag="ddt4")
            seg_e = []
            epb_e = []
            gram4 = decp.tile([128, NCPB * 128], BF16, tag="gram4")
            bln4 = decp.tile([128, NCPB * 128], BF16, tag="bln4")
            for cl in range(NCPB):
                csl = slice(cl * CHUNK, (cl + 1) * CHUNK)
                hsl4 = slice(cl * HL, (cl + 1) * HL)

                # small transposes: csT, dtT (fp32 via f32r identity)
                pT = psS.tile([128, 512], F32, tag="small")
                nc.tensor.transpose(pT[:, :HL], cs[:, csl],
                                    id_sb[:HL, :HL])
                nc.vector.tensor_copy(csT4[:, hsl4], pT[:, :HL])
                nc.vector.tensor_scalar(negcsT4[:, hsl4], pT[:, :HL], -1.0,
                                        None, ALU.mult)
                pT2 = psS.tile([128, 512], F32, tag="small")
                nc.tensor.transpose(pT2[:, :HL], dtsp[:, csl],
                                    id_sb[:HL, :HL])
                nc.vector.tensor_copy(dtT4[:, hsl4], pT2[:, :HL])

                # cs at chunk end, broadcast across partitions (PE matmul)
                pT3 = psS.tile([128, 512], F32, tag="small")
                e127b = bass.AP(tensor=e127_sb.tensor,
                                offset=e127_sb[:].offset,
                                ap=[[e127_sb[:].ap[0][0], 128], [0, 128]])
                nc.tensor.matmul(pT3[:, :HL], e127b, csT4[:, hsl4],
                                 start=True, stop=True)
                nc.scalar.activation(cdbc4[:, hsl4], pT3[:, :HL], AF.Exp)
                nc.vector.tensor_tensor(pT3[:, :HL], pT3[:, :HL],
                                        csT4[:, hsl4], ALU.subtract)
                dec_t = psS.tile([128, 512], F32, tag="small")
                nc.scalar.activation(dec_t[:, :HL], pT3[:, :HL], AF.Exp)
                nc.vector.tensor_tensor(ddt4[:, hsl4], dtT4[:, hsl4],
                                        dec_t[:, :HL], ALU.mult)

                # B chunk transposed (B_LN), bf16
                pbt = psT.tile([128, 1024], BF16, tag="trans")
                nc.tensor.transpose(pbt[:, :128],
                                    convo[:, 8 * SB + cl * CHUNK:
                                          8 * SB + (cl + 1) * CHUNK],
                                    idb_sb[:])
                nc.vector.tensor_copy(bln4[:, cl * 128:(cl + 1) * 128],
                                      pbt[:, :128])

                # Gram^T = B C^T in [s, l]; bf16
                gram_ps = psS.tile([128, 512], F32, tag="small")
                nc.tensor.matmul(
                    gram_ps[:, :128],
                    convo[:, 8 * SB + cl * CHUNK:8 * SB + (cl + 1) * CHUNK],
                    convo[:, 9 * SB + cl * CHUNK:9 * SB + (cl + 1) * CHUNK],
                    start=True, stop=True)
                nc.vector.tensor_copy(gram4[:, cl * 128:(cl + 1) * 128],
                                      gram_ps[:, :128])

                # per-head decay matrices in groups of 4 heads
                sege = decp.tile([128, HL * 128], BF16, tag=f"sege{cl}")
                epbe = decp.tile([128, HL * 128], BF16, tag=f"epbe{cl}")
                seg_e.append(sege)
                epb_e.append(epbe)
                for g in range(HL // 4):
                    pb4 = psS.tile([128, 512], F32, tag="small")
                    for j in range(4):
                        h = 4 * g + j
                        idcol = id_sb[:HL, h:h + 1]
                        indh = bass.AP(tensor=idcol.tensor,
                                       offset=idcol.offset,
                                       ap=[[idcol.ap[0][0], HL], [0, 128]])
                        nc.tensor.matmul(pb4[:, j * 128:(j + 1) * 128], indh,
                                         cs[:, csl], start=True, stop=True)
                    gsl = slice(g * 512, (g + 1) * 512)
                    # epb = exp(pb4) * C (bf16)
                    nc.scalar.activation(epbe[:, gsl], pb4[:], AF.Exp)
                    # seg = exp(pb4 - csT + negmask) * gram (bf16)
                    for j in range(4):
                        h = 4 * g + j
                        nc.vector.scalar_tensor_tensor(
                            sege[:, g * 512 + j * 128:g * 512 + (j + 1) * 128],
                            pb4[:, j * 128:(j + 1) * 128],
                            negcsT4[:, cl * HL + h:cl * HL + h + 1],
                            nm_sb[:], ALU.add, ALU.add)
                    nc.scalar.activation(sege[:, gsl], sege[:, gsl], AF.Exp)
                    gram_c = gram4[:, cl * 128:(cl + 1) * 128]
                    gram_b = bass.AP(tensor=gram_c.tensor, offset=gram_c.offset,
                                     ap=[gram_c.ap[0], [0, 4], [1, 128]])
                    ccs_c = convo[:, 9 * SB + cl * CHUNK:9 * SB + (cl + 1) * CHUNK]
                    ccs_b = bass.AP(tensor=ccs_c.tensor, offset=ccs_c.offset,
                                    ap=[ccs_c.ap[0], [0, 4], [1, 128]])
                    s4 = sege[:, gsl].rearrange("p (j l) -> p j l", j=4)
                    e4 = epbe[:, gsl].rearrange("p (j l) -> p j l", j=4)
                    nc.vector.tensor_tensor(s4, s4, gram_b, ALU.mult)
                    nc.vector.tensor_tensor(e4, e4, ccs_b, ALU.mult)
                drain_out(3)

            # ---------------- SSD chunks (copy/square only) -------------
            qstage = gqp.tile([128, NK2 * SB], BF16, tag="gq")
            ysbs = []
            ssum4 = ch1p.tile([128, NCPB], F32, tag="ssum4")

            for cl in range(NCPB):
                sege, epbe = seg_e[cl], epb_e[cl]
                y_sb = ch1p.tile([128, CLOC], BF16, tag=f"ysb{cl}")
                ysbs.append(y_sb)

                def bc16(tile4, nh, cl=cl, h0=0):
                    sl = tile4[:, cl * HL + h0:cl * HL + h0 + nh]
                    return bass.AP(tensor=sl.tensor, offset=sl.offset,
                                   ap=[sl.ap[0], [sl.ap[-1][0], nh], [0, HD]])

                # process 8 heads (512 cols) at a time to halve PSUM tiles
                for half in range(2):
                    h0 = half * (HL // 2)
                    hsl = slice(half * 512, (half + 1) * 512)
                    # gate+x transposes into one bf16 PSUM tile
                    gx = psT.tile([128, 1024], BF16, tag="trans")
                    for t in range(4):
                        tt = half * 4 + t
                        src = slice(tt * SB + cl * CHUNK,
                                    tt * SB + (cl + 1) * CHUNK)
                        nc.tensor.transpose(
                            gx[:, t * 128:(t + 1) * 128],
                            gate_sb[:, src], idb_sb[:])
                        nc.tensor.transpose(
                            gx[:, 512 + t * 128:512 + (t + 1) * 128],
                            convo[:, src], idb_sb[:])
                    sgx = ch1p.tile([128, 1024], BF16, tag=f"sgx{half}")
                    nc.vector.tensor_copy(sgx[:], gx[:])
                    silg = sgx[:, :512]
                    xT = sgx[:, 512:]
                    xdt = ch1p.tile([128, 512], BF16, tag=f"xdt{half}")
                    xdd = ch1p.tile([128, 512], BF16, tag=f"xdd{half}")
                    x3 = xT.rearrange("p (h d) -> p h d", h=HL // 2)
                    nc.vector.tensor_tensor(
                        xdt[:].rearrange("p (h d) -> p h d", h=HL // 2),
                        x3, bc16(dtT4, HL // 2, h0=h0), ALU.mult)
                    nc.vector.tensor_tensor(
                        xdd[:].rearrange("p (h d) -> p h d", h=HL // 2),
                        x3, bc16(ddt4, HL // 2, h0=h0), ALU.mult)

                    # y = scores@xdt + epb@st + D*x (one PSUM group per head)
                    y_ps = psY.tile([128, 512], F32, tag="yo")
                    for hh in range(HL // 2):
                        h = h0 + hh
                        hs = slice(hh * HD, (hh + 1) * HD)
                        hsg = slice(h * HD, (h + 1) * HD)
                        nc.tensor.matmul(
                            y_ps[:, hs], sege[:, h * 128:(h + 1) * 128],
                            xdt[:, hs], start=True, stop=False)
                        nc.tensor.matmul(
                            y_ps[:, hs], epbe[:, h * 128:(h + 1) * 128],
                            st_sb[:, hsg], start=False, stop=False)
                        nc.tensor.matmul(
                            y_ps[:, hs], ddiag_sb[:, h * 128:(h + 1) * 128],
                            xT[:, hs], start=False, stop=True)

                    # states for these 8 heads
                    s_ps = psY.tile([128, 512], F32, tag="yo")
                    nc.tensor.matmul(
                        s_ps[:], bln4[:, cl * 128:(cl + 1) * 128],
                        xdd[:], start=True, stop=True)

                    # state update: st = st*cdbc + s_ps (bf16)
                    sttmp = ch1p.tile([128, 512], BF16, tag=f"stt{half}")
                    st3 = st_sb[:, hsl].rearrange("p (h d) -> p h d",
                                                  h=HL // 2)
                    nc.vector.tensor_tensor(
                        sttmp[:].rearrange("p (h d) -> p h d", h=HL // 2),
                        st3, bc16(cdbc4, HL // 2, h0=h0), ALU.mult)
                    nc.vector.tensor_tensor(st_sb[:, hsl], sttmp[:],
                                            s_ps[:], ALU.add)

                    # gate: y_sb = y_ps * silg (bf16)
                    nc.vector.tensor_tensor(y_sb[:, hsl], y_ps[:], silg,
                                            ALU.mult)
                sq = ch1p.tile([128, CLOC], BF16, tag="sqscr")
                nc.scalar.activation(sq[:], y_sb[:], AF.Square,
                                     accum_out=ssum4[:, cl:cl + 1])
                drain_out(3)

            # ------------- norm + stage (sqrt table once) ---------------
            nc.vector.tensor_scalar(ssum4[:], ssum4[:], 1.0 / GROUP, EPS,
                                    ALU.mult, ALU.add)
            rstd4 = chp.tile([128, NCPB], F32, tag="rstd4")
            nc.scalar.activation(rstd4[:], ssum4[:], AF.Sqrt)
            nc.vector.reciprocal(rstd4[:], rstd4[:])
            for cl in range(NCPB):
                normed = ch1p.tile([128, CLOC], BF16, tag="normed")
                nc.vector.tensor_scalar(normed[:], ysbs[cl][:],
                                        rstd4[:, cl:cl + 1], None, ALU.mult)
                nps = psT.tile([128, 1024], BF16, tag="trans")
                for t in range(8):
                    nc.tensor.transpose(
                        nps[:, t * 128:(t + 1) * 128],
                        normed[:, t * 128:(t + 1) * 128], idb_sb[:])
                qdst = qstage[:].rearrange(
                    "p (t s) -> p t s", t=NK2)[:, :, cl * 128:(cl + 1) * 128]
                nsrc = nps[:].rearrange("p (t s) -> p t s", t=NK2)
                nc.scalar.copy(qdst, nsrc)
                drain_out(1)

            # out_proj m-blocks are deferred and interleaved into the
            # next superblock's in_proj f-loop (shared psA rotation)
            pending_out.extend((m, qstage, sb) for m in range(NM2))

        while pending_out:
            emit_outproj(*pending_out.pop(0))


def prepare_in_maps(hidden_states, in_proj_w, conv_w, conv_b, dt_bias, D,
                    norm_w, out_proj_w):
    hidT = np.ascontiguousarray(hidden_states.reshape(S, H_SIZE).T)
    # [half, kk, r, sb, c] -> [sb, half, r, kk, c]
    hids = np.ascontiguousarray(
        hidT.reshape(2, 16, 128, NSB, SB).transpose(3, 0, 2, 1, 4)
        .reshape(NSB, 2, 128, 16 * SB)).astype(NPBF)
    negmask = np.where(np.arange(128)[None, :] >= np.arange(128)[:, None],
                       np.float32(0.0), np.float32(NEGM)).astype(np.float32)
    ident = np.eye(128, dtype=np.float32)
    identb = np.eye(128, dtype=np.float32).astype(NPBF)
    e127 = np.zeros((128, 1), np.float32)
    e127[127, 0] = 1.0
    in_maps = []
    for c in range(N_CORES):
        gsl = slice(CLOC * c, CLOC * (c + 1))
        xsl = slice(INTER + CLOC * c, INTER + CLOC * (c + 1))
        bsl = slice(2 * INTER + SS * c, 2 * INTER + SS * (c + 1))
        cslc = slice(2 * INTER + NG * SS + SS * c,
                     2 * INTER + NG * SS + SS * (c + 1))
        dsl = slice(INTER + CONV_DIM + HL * c, INTER + CONV_DIM + HL * (c + 1))
        w1 = np.concatenate([in_proj_w[gsl], in_proj_w[xsl], in_proj_w[bsl],
                             in_proj_w[cslc], in_proj_w[dsl]], axis=0)
        w1 = np.concatenate(
            [w1, np.zeros((NF * 128 - w1.shape[0], H_SIZE), np.float32)],
            axis=0)
        # W1T [4096, 2432]: [half, kk, r, f, fc] -> [f, half, r, kk, fc]
        w1f = np.ascontiguousarray(
            w1.T.reshape(2, 16, 128, NF, 128).transpose(3, 0, 2, 1, 4)
            .reshape(NF, 2, 128, 16 * 128)).astype(NPBF)
        w2 = out_proj_w[:, gsl] * norm_w[gsl][None, :]  # norm_w folded
        # W2T [1024, 4096]: [kt, r, m, mc] -> [m, r, kt, mc]
        w2m = np.ascontiguousarray(
            w2.T.reshape(NK2, 128, NM2, 128).transpose(2, 1, 0, 3)
            .reshape(NM2, 128, NK2 * 128)).astype(NPBF)
        conv_idx = np.concatenate([
            np.arange(CLOC * c, CLOC * (c + 1)),
            np.arange(INTER + SS * c, INTER + SS * (c + 1)),
            np.arange(INTER + NG * SS + SS * c,
                      INTER + NG * SS + SS * (c + 1))])
        cwl = conv_w[conv_idx, 0, :]          # [1280, 4]
        cbl = conv_b[conv_idx]                # [1280]
        convw = np.ascontiguousarray(
            cwl.reshape(10, 128, KCONV).transpose(1, 0, 2)
            .reshape(128, 10 * KCONV))
        convb = np.ascontiguousarray(cbl.reshape(10, 128).transpose(1, 0))
        hsl = slice(HL * c, HL * (c + 1))
        acol = -(np.arange(HL * c + 1, HL * (c + 1) + 1, dtype=np.float32))
        ddiag = np.zeros((128, HL * 128), np.float32)
        for h in range(HL):
            ddiag[:, h * 128:(h + 1) * 128] = np.eye(128) * D[HL * c + h]
        in_maps.append({
            "hids": hids,
            "w1f": w1f,
            "w2m": w2m,
            "convw": convw,
            "convb": convb,
            "dtbias": dt_bias[hsl].reshape(HL, 1).astype(np.float32),
            "acol": acol.reshape(HL, 1),
            "ddiag": ddiag.astype(NPBF),
            "negmask": negmask,
            "ident": ident,
            "identb": identb,
            "e127": e127,
        })
    return in_maps


def get_nc():
    if "nc" not in _CACHE:
        _CACHE["nc"] = build_nc()
    return _CACHE["nc"]


def kernel(hidden_states, in_proj_w, conv_w, conv_b, dt_bias, D, norm_w,
           out_proj_w):
    nc = get_nc()
    in_maps = prepare_in_maps(
        np.asarray(hidden_states, np.float32),
        np.asarray(in_proj_w, np.float32),
        np.asarray(conv_w, np.float32), np.asarray(conv_b, np.float32),
        np.asarray(dt_bias, np.float32), np.asarray(D, np.float32),
        np.asarray(norm_w, np.float32), np.asarray(out_proj_w, np.float32))
    res = run_bass_kernel_spmd(nc, in_maps, list(range(N_CORES)))
    acc = np.zeros((H_SIZE, S), np.float64)
    for r in res.results:
        acc += r["outp"].astype(np.float32).transpose(0, 2, 1, 3).reshape(
            H_SIZE, S)
    return acc.T.astype(np.float32).reshape(1, S, H_SIZE)


# revision 3
# speedup vs baseline: 1.2899x; 1.0574x over previous
"""NemotronH Mamba2 mixer on 8 Trainium2 cores (Bass/Tile) — v2 (bf16).

Sharding: tensor-parallel over heads/groups. Core c owns group c =
16 heads (= 1024 gate/x channels, 128 B + 128 C state channels, 16 dt).
out_proj is row-parallel over the contraction; bf16 partials are
combined on the host.

v2 changes vs baseline: bf16 matmuls/transposes/elementwise everywhere
except the dt/cs chain (fp32), activation-table phasing (silu -> exp ->
sqrt once per superblock), conv taps at superblock granularity (4x DVE
mode), decay matrices built in a pre-pass with batched exps, D*x folded
into the per-head PSUM group via constant D*I stationary tiles.
"""

import numpy as np
import ml_dtypes

import concourse.bass as bass
import concourse.mybir as mybir
from concourse import bacc
from concourse.tile import TileContext
from concourse.bass_utils import run_bass_kernel_spmd

F32 = mybir.dt.float32
F32R = mybir.dt.float32r
BF16 = mybir.dt.bfloat16
F8 = mybir.dt.float8e4
DR = mybir.MatmulPerfMode.DoubleRow
AF = mybir.ActivationFunctionType
ALU = mybir.AluOpType
NPBF = ml_dtypes.bfloat16
NP8 = ml_dtypes.float8_e4m3

# fp8 scales (powers of two; residuals stored in the same scaled units)
SX = 32.0      # hidden_states
SW = 1024.0    # in_proj weights
DEQ1 = 1.0 / (SX * SW)
QS = 4.0       # on-chip normed quantization
SW2 = 1024.0   # out_proj weights
DEQ2 = 1.0 / (QS * SW2)

# Model dims
H_SIZE = 4096
NH = 128
HD = 64
SS = 128
KCONV = 4
NG = 8
CHUNK = 128
INTER = NH * HD                 # 8192
CONV_DIM = INTER + 2 * NG * SS  # 10240
PROJ = INTER + CONV_DIM + NH    # 18560
DT_MIN, DT_MAX = 0.001, 100.0
EPS = 1e-5
GROUP = INTER // NG             # 1024

# Sharding / tiling
N_CORES = 8
S = 2048
HL = NH // N_CORES              # 16 local heads
CLOC = HL * HD                  # 1024 local gate/x channels
NSB = 4                         # seq superblocks
SB = S // NSB                   # 512
NCPB = SB // CHUNK              # 4 chunks per superblock
NF = 19                         # in_proj f-tiles (2432 = 19*128, padded)
NK1 = H_SIZE // 128             # 32 k-tiles for in_proj
NK2 = CLOC // 128               # 8 k-tiles for out_proj
NM2 = H_SIZE // 128             # 32 m-tiles for out_proj
NEGM = -1e30

# log1p(u)/u on [0,1], Chebyshev-fit degree 12 (max fp32 err ~1.1e-7)
LOG1P_C = [0.9999999999815061, -0.4999999935552795, 0.33333295899388315,
           -0.2499913901062215, 0.19989602251462296, -0.1659083573590588,
           0.1392317246686566, -0.1130135727826319, 0.08261769871302305,
           -0.04960969557400616, 0.021956439674455992, -0.006180556818034449,
           0.0008159022224092772]

_CACHE = {}


def r32(ap):
    return ap.bitcast(F32R)


def build_nc():
    nc = bacc.Bacc(None, target_bir_lowering=False)

    # hidden, pre-tiled: [sb, half, 128, 16*512] fp8 hi + lo residual
    hids = nc.declare_dram_parameter("hids", [NSB, 2, 128, 16 * SB], F8,
                                     isOutput=False)
    hidsl = nc.declare_dram_parameter("hidsl", [NSB, 2, 128, 16 * SB], F8,
                                      isOutput=False)
    # in_proj weights, pre-tiled per f-tile: [f, half, 128, 16*128] fp8 hi+lo
    w1f = nc.declare_dram_parameter("w1f", [NF, 2, 128, 16 * 128], F8,
                                    isOutput=False)
    w1fl = nc.declare_dram_parameter("w1fl", [NF, 2, 128, 16 * 128], F8,
                                     isOutput=False)
    # out_proj weights, pre-tiled per m-tile: [m, 128, 8*128] fp8 hi+lo
    w2m = nc.declare_dram_parameter("w2m", [NM2, 128, NK2 * 128], F8,
                                    isOutput=False)
    w2ml = nc.declare_dram_parameter("w2ml", [NM2, 128, NK2 * 128], F8,
                                     isOutput=False)
    convw = nc.declare_dram_parameter("convw", [128, 10 * KCONV], F32,
                                      isOutput=False)
    convb = nc.declare_dram_parameter("convb", [128, 10], F32, isOutput=False)
    dtbias = nc.declare_dram_parameter("dtbias", [HL, 1], F32, isOutput=False)
    acol = nc.declare_dram_parameter("acol", [HL, 1], F32, isOutput=False)
    ddiag = nc.declare_dram_parameter("ddiag", [128, HL * 128], BF16,
                                      isOutput=False)
    negmask = nc.declare_dram_parameter("negmask", [128, 128], F32,
                                        isOutput=False)
    ident = nc.declare_dram_parameter("ident", [128, 128], F32, isOutput=False)
    identb = nc.declare_dram_parameter("identb", [128, 128], BF16,
                                       isOutput=False)
    e127 = nc.declare_dram_parameter("e127", [128, 1], F32, isOutput=False)
    outp = nc.declare_dram_parameter("outp", [NM2, 4, 128, 512], BF16,
                                     isOutput=True)

    with TileContext(nc) as tc:
        with tc.tile_pool(name="const", bufs=1) as cp:
            id_sb = cp.tile([128, 128], F32, tag="id")
            idb_sb = cp.tile([128, 128], BF16, tag="idb")
            nm_sb = cp.tile([128, 128], F32, tag="nm")
            ddiag_sb = cp.tile([128, HL * 128], BF16, tag="ddiag")
            cw_sb = cp.tile([128, 10 * KCONV], F32, tag="cw")
            cb_sb = cp.tile([128, 10], F32, tag="cb")
            dtb_sb = cp.tile([HL, 1], F32, tag="dtb")
            a_sb = cp.tile([HL, 1], F32, tag="acol")
            e127_sb = cp.tile([128, 1], F32, tag="e127")
            ones16 = cp.tile([HL, CHUNK], F32, tag="ones16")
            st_sb = cp.tile([128, HL * HD], BF16, tag="state")
            nc.sync.dma_start(out=id_sb[:], in_=ident[:])
            nc.sync.dma_start(out=idb_sb[:], in_=identb[:])
            nc.sync.dma_start(out=nm_sb[:], in_=negmask[:])
            nc.sync.dma_start(out=ddiag_sb[:], in_=ddiag[:])
            nc.sync.dma_start(out=cw_sb[:], in_=convw[:])
            nc.sync.dma_start(out=cb_sb[:], in_=convb[:])
            nc.sync.dma_start(out=dtb_sb[:], in_=dtbias[:])
            nc.sync.dma_start(out=a_sb[:], in_=acol[:])
            nc.sync.dma_start(out=e127_sb[:], in_=e127[:])
            nc.vector.memset(ones16[:], 1.0)
            nc.vector.memset(st_sb[:], 0.0)

            _main_phase(nc, tc, (hids, hidsl), (w1f, w1fl), id_sb, idb_sb,
                        nm_sb, ddiag_sb, cw_sb, cb_sb, dtb_sb, a_sb, e127_sb,
                        ones16, st_sb, (w2m, w2ml), outp)

    nc.compile()
    return nc


def _main_phase(nc, tc, hids2, w1f2, id_sb, idb_sb, nm_sb, ddiag_sb,
                cw_sb, cb_sb, dtb_sb, a_sb, e127_sb, ones16,
                st_sb, w2m2, outp):
    hids, hidsl = hids2
    w1f, w1fl = w1f2
    w2m, w2ml = w2m2
    with tc.tile_pool(name="hid", bufs=2) as hidp, \
         tc.tile_pool(name="w1", bufs=3) as w1p, \
         tc.tile_pool(name="gq", bufs=2) as gqp, \
         tc.tile_pool(name="conv", bufs=1) as convp, \
         tc.tile_pool(name="dtl", bufs=1) as dtp, \
         tc.tile_pool(name="dtr", bufs=2) as dtrp, \
         tc.tile_pool(name="dec", bufs=1) as decp, \
         tc.tile_pool(name="chunk", bufs=2) as chp, \
         tc.tile_pool(name="chunk1", bufs=1) as ch1p, \
         tc.tile_pool(name="w2", bufs=2) as w2p, \
         tc.tile_pool(name="oev", bufs=2) as oevp, \
         tc.tile_pool(name="psA", bufs=2, space="PSUM") as psA, \
         tc.tile_pool(name="psS", bufs=2, space="PSUM") as psS, \
         tc.tile_pool(name="psT", bufs=2, space="PSUM") as psT, \
         tc.tile_pool(name="psY", bufs=2, space="PSUM") as psY:

        # convcat: 10 conv channel tiles (8 x, 1 B, 1 C), each 3 halo + 512
        ccat = convp.tile([128, 10 * (SB + 3)], BF16, tag="ccat")
        for t in range(10):
            nc.vector.memset(ccat[:, t * (SB + 3):t * (SB + 3) + 3], 0.0)
        # conv+silu output: x tiles 0..7, B tile 8, C tile 9
        convo = convp.tile([128, 10 * SB], BF16, tag="convo")

        pending_out = []

        def emit_outproj(m, qst2, sbq):
            qh, ql = qst2
            w2 = w2p.tile([128, NK2 * 128], F8, tag="w2")
            w2l = w2p.tile([128, NK2 * 128], F8, tag="w2l")
            nc.sync.dma_start(out=w2[:], in_=w2m[m])
            nc.sync.dma_start(out=w2l[:], in_=w2ml[m])
            acc = psA.tile([128, 512], F32, tag="ipacc")
            w3 = w2[:].rearrange("p (k c) -> p k c", k=NK2)
            w3l = w2l[:].rearrange("p (k c) -> p k c", k=NK2)
            q3 = qh[:].rearrange("p (k s) -> p k s", k=NK2)
            q3l = ql[:].rearrange("p (k s) -> p k s", k=NK2)
            npair = NK2 // 2
            for p in range(npair):
                ksl = slice(2 * p, 2 * p + 2)
                nc.tensor.matmul(acc[:], w3[:, ksl], q3[:, ksl],
                                 start=(p == 0), stop=False, perf_mode=DR)
                nc.tensor.matmul(acc[:], w3l[:, ksl], q3[:, ksl],
                                 start=False, stop=False, perf_mode=DR)
                nc.tensor.matmul(acc[:], w3[:, ksl], q3l[:, ksl],
                                 start=False, stop=(p == npair - 1),
                                 perf_mode=DR)
            ev = oevp.tile([128, 512], BF16, tag="oev")
            nc.scalar.copy(ev[:], acc[:])
            nc.sync.dma_start(out=outp[m, sbq], in_=ev[:])

        def drain_out(n):
            for _ in range(n):
                if pending_out:
                    emit_outproj(*pending_out.pop(0))

        for sb in range(NSB):
            # ---------------- in_proj for this superblock ----------------
            halves, halvesl = [], []
            for khalf in range(2):
                hid_h = hidp.tile([128, 16 * SB], F8, tag="hid")
                nc.sync.dma_start(out=hid_h[:], in_=hids[sb, khalf])
                halves.append(hid_h)
                hid_l = hidp.tile([128, 16 * SB], F8, tag="hidl")
                nc.sync.dma_start(out=hid_l[:], in_=hidsl[sb, khalf])
                halvesl.append(hid_l)

            gate_sb = gqp.tile([128, 8 * SB], BF16, tag="gq")
            dtraw = dtrp.tile([HL, SB], F32, tag="dtraw")

            # halo copies must read previous superblock before overwrite
            if sb > 0:
                for t in range(10):
                    base = t * (SB + 3)
                    nc.vector.tensor_copy(
                        ccat[:, base:base + 3], ccat[:, base + SB:base + SB + 3])

            def emit_ftile(f, gate_sb=gate_sb, dtraw=dtraw,
                           halves=halves, halvesl=halvesl):
                w1h, w1hl = [], []
                for khalf in range(2):
                    w1t_ = w1p.tile([128, 16 * 128], F8, tag="w1")
                    nc.sync.dma_start(out=w1t_[:], in_=w1f[f, khalf])
                    w1h.append(w1t_)
                    w1l_ = w1p.tile([128, 16 * 128], F8, tag="w1l")
                    nc.sync.dma_start(out=w1l_[:], in_=w1fl[f, khalf])
                    w1hl.append(w1l_)
                acc = psA.tile([128, SB], F32, tag="ipacc")
                for khalf in range(2):
                    w3 = w1h[khalf][:].rearrange("p (k c) -> p k c", k=16)
                    w3l = w1hl[khalf][:].rearrange("p (k c) -> p k c", k=16)
                    h3 = halves[khalf][:].rearrange("p (k s) -> p k s", k=16)
                    h3l = halvesl[khalf][:].rearrange("p (k s) -> p k s", k=16)
                    for p in range(8):
                        ksl = slice(2 * p, 2 * p + 2)
                        nc.tensor.matmul(
                            acc[:], w3[:, ksl], h3[:, ksl],
                            start=(khalf == 0 and p == 0), stop=False,
                            perf_mode=DR)
                        nc.tensor.matmul(
                            acc[:], w3l[:, ksl], h3[:, ksl],
                            start=False, stop=False, perf_mode=DR)
                        nc.tensor.matmul(
                            acc[:], w3[:, ksl], h3l[:, ksl],
                            start=False, stop=(khalf == 1 and p == 7),
                            perf_mode=DR)
                if f < 8:
                    # fused dequant + SiLU at eviction -> bf16 gate
                    nc.scalar.activation(
                        gate_sb[:, f * SB:(f + 1) * SB], acc[:], AF.Silu,
                        scale=DEQ1)
                elif f < 18:
                    t = f - 8
                    base = t * (SB + 3)
                    nc.scalar.activation(
                        ccat[:, base + 3:base + 3 + SB], acc[:], AF.Identity,
                        scale=DEQ1)
                else:
                    nc.scalar.activation(dtraw[:, :], acc[:HL, :],
                                         AF.Identity, scale=DEQ1)

            for f in [18] + list(range(18)):
                emit_ftile(f)

            # ---------------- conv (DVE, bf16 4x) + SiLU ----------------
            for t in range(10):
                base = t * (SB + 3)
                dst = convo[:, t * SB:(t + 1) * SB]
                nc.vector.tensor_scalar(
                    dst, ccat[:, base:base + SB],
                    cw_sb[:, t * KCONV:t * KCONV + 1], cb_sb[:, t:t + 1],
                    ALU.mult, ALU.add)
                for j in range(1, KCONV):
                    nc.vector.scalar_tensor_tensor(
                        dst, ccat[:, base + j:base + j + SB],
                        cw_sb[:, t * KCONV + j:t * KCONV + j + 1], dst,
                        ALU.mult, ALU.add)
            nc.scalar.activation(convo[:, :8 * SB], convo[:, :8 * SB], AF.Silu)
            nc.scalar.activation(convo[:, 8 * SB:], convo[:, 8 * SB:], AF.Silu)
            drain_out(2)

            # ---------------- dt pipeline (fp32, exp table from here) ----
            # softplus(z) = relu(z) + log1p(exp(-|z|)); log1p via poly
            uu = dtp.tile([HL, SB], F32, tag="uu")
            pp = dtrp.tile([HL, SB], F32, tag="pp")
            dtsp = dtraw  # in-place: relu(z) overwrites z
            cs = pp       # reuse pp once the poly is folded in
            nc.scalar.activation(dtraw[:], dtraw[:], AF.Identity,
                                 bias=dtb_sb[:, 0:1])
            # uu = exp(min(z, -z)) = exp(-|z|)
            nc.vector.tensor_scalar(uu[:], dtraw[:], -1.0, None, ALU.mult)
            nc.vector.tensor_tensor(uu[:], uu[:], dtraw[:], ALU.min)
            nc.scalar.activation(uu[:], uu[:], AF.Exp)
            # Horner for q(u) = log1p(u)/u
            nc.vector.tensor_scalar(pp[:], uu[:], LOG1P_C[-1], LOG1P_C[-2],
                                    ALU.mult, ALU.add)
            for cidx in range(len(LOG1P_C) - 3, -1, -1):
                nc.vector.tensor_tensor(pp[:], pp[:], uu[:], ALU.mult)
                nc.vector.tensor_scalar(pp[:], pp[:], LOG1P_C[cidx], None,
                                        ALU.add)
            nc.vector.tensor_tensor(pp[:], pp[:], uu[:], ALU.mult)
            relu_t = uu  # uu dead; use as relu scratch
            nc.scalar.activation(relu_t[:], dtraw[:], AF.Relu)
            nc.vector.tensor_tensor(dtsp[:], relu_t[:], pp[:], ALU.add)
            nc.vector.tensor_scalar(dtsp[:], dtsp[:], DT_MIN, DT_MAX,
                                    ALU.max, ALU.min)
            dA = uu  # reuse again (relu scratch is dead)
            nc.vector.tensor_scalar(dA[:], dtsp[:], a_sb[:, 0:1], None,
                                    ALU.mult)
            for cl in range(NCPB):
                nc.vector.tensor_tensor_scan(
                    cs[:, cl * CHUNK:(cl + 1) * CHUNK],
                    ones16[:], dA[:, cl * CHUNK:(cl + 1) * CHUNK],
                    0.0, ALU.mult, ALU.add)
            drain_out(2)

            # ------------ decay pre-pass for all 4 chunks (exp table) ----
            # per-chunk tiles, all bf16 except the fp32 cs-derived smalls
            csT4 = decp.tile([128, NCPB * HL], F32, tag="csT4")
            negcsT4 = decp.tile([128, NCPB * HL], F32, tag="negcsT4")
            dtT4 = decp.tile([128, NCPB * HL], F32, tag="dtT4")
            cdbc4 = decp.tile([128, NCPB * HL], F32, tag="cdbc4")
            ddt4 = decp.tile([128, NCPB * HL], F32, tag="ddt4")
            seg_e = []
            epb_e = []
            gram4 = decp.tile([128, NCPB * 128], BF16, tag="gram4")
            bln4 = decp.tile([128, NCPB * 128], BF16, tag="bln4")
            for cl in range(NCPB):
                csl = slice(cl * CHUNK, (cl + 1) * CHUNK)
                hsl4 = slice(cl * HL, (cl + 1) * HL)

                # small transposes: csT, dtT (fp32 via f32r identity)
                pT = psS.tile([128, 512], F32, tag="small")
                nc.tensor.transpose(pT[:, :HL], cs[:, csl],
                                    id_sb[:HL, :HL])
                nc.vector.tensor_copy(csT4[:, hsl4], pT[:, :HL])
                nc.vector.tensor_scalar(negcsT4[:, hsl4], pT[:, :HL], -1.0,
                                        None, ALU.mult)
                pT2 = psS.tile([128, 512], F32, tag="small")
                nc.tensor.transpose(pT2[:, :HL], dtsp[:, csl],
                                    id_sb[:HL, :HL])
                nc.vector.tensor_copy(dtT4[:, hsl4], pT2[:, :HL])

                # cs at chunk end, broadcast across partitions (PE matmul)
                pT3 = psS.tile([128, 512], F32, tag="small")
                e127b = bass.AP(tensor=e127_sb.tensor,
                                offset=e127_sb[:].offset,
                                ap=[[e127_sb[:].ap[0][0], 128], [0, 128]])
                nc.tensor.matmul(pT3[:, :HL], e127b, csT4[:, hsl4],
                                 start=True, stop=True)
                nc.scalar.activation(cdbc4[:, hsl4], pT3[:, :HL], AF.Exp)
                nc.vector.tensor_tensor(pT3[:, :HL], pT3[:, :HL],
                                        csT4[:, hsl4], ALU.subtract)
                dec_t = psS.tile([128, 512], F32, tag="small")
                nc.scalar.activation(dec_t[:, :HL], pT3[:, :HL], AF.Exp)
                nc.vector.tensor_tensor(ddt4[:, hsl4], dtT4[:, hsl4],
                                        dec_t[:, :HL], ALU.mult)

                # B chunk transposed (B_LN), bf16
                pbt = psT.tile([128, 1024], BF16, tag="trans")
                nc.tensor.transpose(pbt[:, :128],
                                    convo[:, 8 * SB + cl * CHUNK:
                                          8 * SB + (cl + 1) * CHUNK],
                                    idb_sb[:])
                nc.vector.tensor_copy(bln4[:, cl * 128:(cl + 1) * 128],
                                      pbt[:, :128])

                # Gram^T = B C^T in [s, l]; bf16
                gram_ps = psS.tile([128, 512], F32, tag="small")
                nc.tensor.matmul(
                    gram_ps[:, :128],
                    convo[:, 8 * SB + cl * CHUNK:8 * SB + (cl + 1) * CHUNK],
                    convo[:, 9 * SB + cl * CHUNK:9 * SB + (cl + 1) * CHUNK],
                    start=True, stop=True)
                nc.vector.tensor_copy(gram4[:, cl * 128:(cl + 1) * 128],
                                      gram_ps[:, :128])

                # per-head decay matrices in groups of 4 heads
                sege = decp.tile([128, HL * 128], BF16, tag=f"sege{cl}")
                epbe = decp.tile([128, HL * 128], BF16, tag=f"epbe{cl}")
                seg_e.append(sege)
                epb_e.append(epbe)
                for g in range(HL // 4):
                    pb4 = psS.tile([128, 512], F32, tag="small")
                    for j in range(4):
                        h = 4 * g + j
                        idcol = id_sb[:HL, h:h + 1]
                        indh = bass.AP(tensor=idcol.tensor,
                                       offset=idcol.offset,
                                       ap=[[idcol.ap[0][0], HL], [0, 128]])
                        nc.tensor.matmul(pb4[:, j * 128:(j + 1) * 128], indh,
                                         cs[:, csl], start=True, stop=True)
                    gsl = slice(g * 512, (g + 1) * 512)
                    # epb = exp(pb4) * C (bf16)
                    nc.scalar.activation(epbe[:, gsl], pb4[:], AF.Exp)
                    # seg = exp(pb4 - csT + negmask) * gram (bf16)
                    for j in range(4):
                        h = 4 * g + j
                        nc.vector.scalar_tensor_tensor(
                            sege[:, g * 512 + j * 128:g * 512 + (j + 1) * 128],
                            pb4[:, j * 128:(j + 1) * 128],
                            negcsT4[:, cl * HL + h:cl * HL + h + 1],
                            nm_sb[:], ALU.add, ALU.add)
                    nc.scalar.activation(sege[:, gsl], sege[:, gsl], AF.Exp)
                    gram_c = gram4[:, cl * 128:(cl + 1) * 128]
                    gram_b = bass.AP(tensor=gram_c.tensor, offset=gram_c.offset,
                                     ap=[gram_c.ap[0], [0, 4], [1, 128]])
                    ccs_c = convo[:, 9 * SB + cl * CHUNK:9 * SB + (cl + 1) * CHUNK]
                    ccs_b = bass.AP(tensor=ccs_c.tensor, offset=ccs_c.offset,
                                    ap=[ccs_c.ap[0], [0, 4], [1, 128]])
                    s4 = sege[:, gsl].rearrange("p (j l) -> p j l", j=4)
                    e4 = epbe[:, gsl].rearrange("p (j l) -> p j l", j=4)
                    nc.vector.tensor_tensor(s4, s4, gram_b, ALU.mult)
                    nc.vector.tensor_tensor(e4, e4, ccs_b, ALU.mult)
                drain_out(3)

            # ---------------- SSD chunks (copy/square only) -------------
            qstage8 = gqp.tile([128, NK2 * SB], F8, tag="q8")
            qstage8l = gqp.tile([128, NK2 * SB], F8, tag="q8l")
            ysbs = []
            ssum4 = ch1p.tile([128, NCPB], F32, tag="ssum4")

            for cl in range(NCPB):
                sege, epbe = seg_e[cl], epb_e[cl]
                y_sb = ch1p.tile([128, CLOC], BF16, tag=f"ysb{cl}")
                ysbs.append(y_sb)

                def bc16(tile4, nh, cl=cl, h0=0):
                    sl = tile4[:, cl * HL + h0:cl * HL + h0 + nh]
                    return bass.AP(tensor=sl.tensor, offset=sl.offset,
                                   ap=[sl.ap[0], [sl.ap[-1][0], nh], [0, HD]])

                # process 8 heads (512 cols) at a time to halve PSUM tiles
                for half in range(2):
                    h0 = half * (HL // 2)
                    hsl = slice(half * 512, (half + 1) * 512)
                    # gate+x transposes into one bf16 PSUM tile
                    gx = psT.tile([128, 1024], BF16, tag="trans")
                    for t in range(4):
                        tt = half * 4 + t
                        src = slice(tt * SB + cl * CHUNK,
                                    tt * SB + (cl + 1) * CHUNK)
                        nc.tensor.transpose(
                            gx[:, t * 128:(t + 1) * 128],
                            gate_sb[:, src], idb_sb[:])
                        nc.tensor.transpose(
                            gx[:, 512 + t * 128:512 + (t + 1) * 128],
                            convo[:, src], idb_sb[:])
                    sgx = ch1p.tile([128, 1024], BF16, tag=f"sgx{half}")
                    nc.vector.tensor_copy(sgx[:], gx[:])
                    silg = sgx[:, :512]
                    xT = sgx[:, 512:]
                    xdt = ch1p.tile([128, 512], BF16, tag=f"xdt{half}")
                    xdd = ch1p.tile([128, 512], BF16, tag=f"xdd{half}")
                    x3 = xT.rearrange("p (h d) -> p h d", h=HL // 2)
                    nc.vector.tensor_tensor(
                        xdt[:].rearrange("p (h d) -> p h d", h=HL // 2),
                        x3, bc16(dtT4, HL // 2, h0=h0), ALU.mult)
                    nc.vector.tensor_tensor(
                        xdd[:].rearrange("p (h d) -> p h d", h=HL // 2),
                        x3, bc16(ddt4, HL // 2, h0=h0), ALU.mult)

                    # y = scores@xdt + epb@st + D*x (one PSUM group per head)
                    y_ps = psY.tile([128, 512], F32, tag="yo")
                    for hh in range(HL // 2):
                        h = h0 + hh
                        hs = slice(hh * HD, (hh + 1) * HD)
                        hsg = slice(h * HD, (h + 1) * HD)
                        nc.tensor.matmul(
                            y_ps[:, hs], sege[:, h * 128:(h + 1) * 128],
                            xdt[:, hs], start=True, stop=False)
                        nc.tensor.matmul(
                            y_ps[:, hs], epbe[:, h * 128:(h + 1) * 128],
                            st_sb[:, hsg], start=False, stop=False)
                        nc.tensor.matmul(
                            y_ps[:, hs], ddiag_sb[:, h * 128:(h + 1) * 128],
                            xT[:, hs], start=False, stop=True)

                    # states for these 8 heads
                    s_ps = psY.tile([128, 512], F32, tag="yo")
                    nc.tensor.matmul(
                        s_ps[:], bln4[:, cl * 128:(cl + 1) * 128],
                        xdd[:], start=True, stop=True)

                    # state update: st = st*cdbc + s_ps (bf16)
                    sttmp = ch1p.tile([128, 512], BF16, tag=f"stt{half}")
                    st3 = st_sb[:, hsl].rearrange("p (h d) -> p h d",
                                                  h=HL // 2)
                    nc.vector.tensor_tensor(
                        sttmp[:].rearrange("p (h d) -> p h d", h=HL // 2),
                        st3, bc16(cdbc4, HL // 2, h0=h0), ALU.mult)
                    nc.vector.tensor_tensor(st_sb[:, hsl], sttmp[:],
                                            s_ps[:], ALU.add)

                    # gate: y_sb = y_ps * silg (bf16)
                    nc.vector.tensor_tensor(y_sb[:, hsl], y_ps[:], silg,
                                            ALU.mult)
                sq = ch1p.tile([128, CLOC], BF16, tag="sqscr")
                nc.scalar.activation(sq[:], y_sb[:], AF.Square,
                                     accum_out=ssum4[:, cl:cl + 1])
                drain_out(3)

            # ------------- norm + stage (sqrt table once) ---------------
            nc.vector.tensor_scalar(ssum4[:], ssum4[:], 1.0 / GROUP, EPS,
                                    ALU.mult, ALU.add)
            rstd4 = chp.tile([128, NCPB], F32, tag="rstd4")
            nc.scalar.activation(rstd4[:], ssum4[:], AF.Sqrt)
            nc.vector.reciprocal(rstd4[:], rstd4[:])
            # fold the fp8 quantization scale into rstd
            nc.vector.tensor_scalar(rstd4[:], rstd4[:], QS, None, ALU.mult)
            for cl in range(NCPB):
                normed = ch1p.tile([128, CLOC], BF16, tag="normed")
                nc.vector.tensor_scalar(normed[:], ysbs[cl][:],
                                        rstd4[:, cl:cl + 1], None, ALU.mult)
                nps = psT.tile([128, 1024], BF16, tag="trans")
                for t in range(8):
                    nc.tensor.transpose(
                        nps[:, t * 128:(t + 1) * 128],
                        normed[:, t * 128:(t + 1) * 128], idb_sb[:])
                csl128 = slice(cl * 128, (cl + 1) * 128)
                qdst = qstage8[:].rearrange(
                    "p (t s) -> p t s", t=NK2)[:, :, csl128]
                qdstl = qstage8l[:].rearrange(
                    "p (t s) -> p t s", t=NK2)[:, :, csl128]
                nsrc = nps[:].rearrange("p (t s) -> p t s", t=NK2)
                nc.scalar.copy(qdst, nsrc)           # fp8 hi
                nc.vector.tensor_tensor(qdstl, nsrc, qdst, ALU.subtract)
                drain_out(1)

            # out_proj m-blocks are deferred and interleaved into the
            # next superblock's in_proj f-loop (shared psA rotation)
            pending_out.extend((m, (qstage8, qstage8l), sb)
                               for m in range(NM2))

        while pending_out:
            emit_outproj(*pending_out.pop(0))


def q8pair(x, scale):
    """fp8 hi + residual lo, both stored in the same scaled units."""
    xs = np.asarray(x, np.float32) * scale
    hi = xs.astype(NP8)
    lo = (xs - hi.astype(np.float32)).astype(NP8)
    return hi, lo


def prepare_in_maps(hidden_states, in_proj_w, conv_w, conv_b, dt_bias, D,
                    norm_w, out_proj_w):
    hidT = np.ascontiguousarray(hidden_states.reshape(S, H_SIZE).T)
    # [half, kk, r, sb, c] -> [sb, half, r, kk, c]
    hids_f = np.ascontiguousarray(
        hidT.reshape(2, 16, 128, NSB, SB).transpose(3, 0, 2, 1, 4)
        .reshape(NSB, 2, 128, 16 * SB))
    hids, hidsl = q8pair(hids_f, SX)
    negmask = np.where(np.arange(128)[None, :] >= np.arange(128)[:, None],
                       np.float32(0.0), np.float32(NEGM)).astype(np.float32)
    ident = np.eye(128, dtype=np.float32)
    identb = np.eye(128, dtype=np.float32).astype(NPBF)
    e127 = np.zeros((128, 1), np.float32)
    e127[127, 0] = 1.0
    in_maps = []
    for c in range(N_CORES):
        gsl = slice(CLOC * c, CLOC * (c + 1))
        xsl = slice(INTER + CLOC * c, INTER + CLOC * (c + 1))
        bsl = slice(2 * INTER + SS * c, 2 * INTER + SS * (c + 1))
        cslc = slice(2 * INTER + NG * SS + SS * c,
                     2 * INTER + NG * SS + SS * (c + 1))
        dsl = slice(INTER + CONV_DIM + HL * c, INTER + CONV_DIM + HL * (c + 1))
        w1 = np.concatenate([in_proj_w[gsl], in_proj_w[xsl], in_proj_w[bsl],
                             in_proj_w[cslc], in_proj_w[dsl]], axis=0)
        w1 = np.concatenate(
            [w1, np.zeros((NF * 128 - w1.shape[0], H_SIZE), np.float32)],
            axis=0)
        # W1T [4096, 2432]: [half, kk, r, f, fc] -> [f, half, r, kk, fc]
        w1f_f = np.ascontiguousarray(
            w1.T.reshape(2, 16, 128, NF, 128).transpose(3, 0, 2, 1, 4)
            .reshape(NF, 2, 128, 16 * 128))
        w1f, w1fl = q8pair(w1f_f, SW)
        w2 = out_proj_w[:, gsl] * norm_w[gsl][None, :]  # norm_w folded
        # W2T [1024, 4096]: [kt, r, m, mc] -> [m, r, kt, mc]
        w2m_f = np.ascontiguousarray(
            w2.T.reshape(NK2, 128, NM2, 128).transpose(2, 1, 0, 3)
            .reshape(NM2, 128, NK2 * 128))
        w2m, w2ml = q8pair(w2m_f, SW2)
        conv_idx = np.concatenate([
            np.arange(CLOC * c, CLOC * (c + 1)),
            np.arange(INTER + SS * c, INTER + SS * (c + 1)),
            np.arange(INTER + NG * SS + SS * c,
                      INTER + NG * SS + SS * (c + 1))])
        cwl = conv_w[conv_idx, 0, :]          # [1280, 4]
        cbl = conv_b[conv_idx]                # [1280]
        convw = np.ascontiguousarray(
            cwl.reshape(10, 128, KCONV).transpose(1, 0, 2)
            .reshape(128, 10 * KCONV))
        convb = np.ascontiguousarray(cbl.reshape(10, 128).transpose(1, 0))
        hsl = slice(HL * c, HL * (c + 1))
        acol = -(np.arange(HL * c + 1, HL * (c + 1) + 1, dtype=np.float32))
        ddiag = np.zeros((128, HL * 128), np.float32)
        for h in range(HL):
            ddiag[:, h * 128:(h + 1) * 128] = np.eye(128) * D[HL * c + h]
        in_maps.append({
            "hids": hids,
            "hidsl": hidsl,
            "w1f": w1f,
            "w1fl": w1fl,
            "w2m": w2m,
            "w2ml": w2ml,
            "convw": convw,
            "convb": convb,
            "dtbias": dt_bias[hsl].reshape(HL, 1).astype(np.float32),
            "acol": acol.reshape(HL, 1),
            "ddiag": ddiag.astype(NPBF),
            "negmask": negmask,
            "ident": ident,
            "identb": identb,
            "e127": e127,
        })
    return in_maps


def get_nc():
    if "nc" not in _CACHE:
        _CACHE["nc"] = build_nc()
    return _CACHE["nc"]


def kernel(hidden_states, in_proj_w, conv_w, conv_b, dt_bias, D, norm_w,
           out_proj_w):
    nc = get_nc()
    in_maps = prepare_in_maps(
        np.asarray(hidden_states, np.float32),
        np.asarray(in_proj_w, np.float32),
        np.asarray(conv_w, np.float32), np.asarray(conv_b, np.float32),
        np.asarray(dt_bias, np.float32), np.asarray(D, np.float32),
        np.asarray(norm_w, np.float32), np.asarray(out_proj_w, np.float32))
    res = run_bass_kernel_spmd(nc, in_maps, list(range(N_CORES)))
    acc = np.zeros((H_SIZE, S), np.float64)
    for r in res.results:
        acc += r["outp"].astype(np.float32).transpose(0, 2, 1, 3).reshape(
            H_SIZE, S)
    acc *= DEQ2  # undo out_proj fp8 scales
    return acc.T.astype(np.float32).reshape(1, S, H_SIZE)
